# revision 1
# baseline (speedup 1.0000x reference)
"""Trainium2 Bass kernel for nn_Attention_43628277793473.

Single-head attention, B=8, S=2048, H=1024:
  q = query @ Wq.T ; k = key @ Wk.T ; v = value @ Wv.T
  score = q @ k.T ; masked_fill(mask==0, -99999) ; softmax ; out = attn @ v

Sharding: data-parallel over the batch dim — one batch element per
NeuronCore (8 cores), no collectives.

Per-core algorithm (all matmuls in float32r: full PE rate at N>=256,
~13 mantissa bits):
  G[a,b]   = sum_o Wq[o,a] Wk[o,b]       # weight gram, natural layouts
  C[a,j]   = sum_b G[a,b] XkT[b,j]       # XkT via PE transpose
  scoreT   = C.T-tiles.T @ XqT           # keys on partitions (flash layout)
  expT     = exp(scoreT + bias_col)      # bias_col = -50000 on masked keys
  ctx[i,:] += expT[:,i].T @ v_j          # accumulated in PSUM over all j
  Z[i]     += expT[:,i].T @ ones         # row sums via N=4 ones matmul
  out[i,:] = ctx[i,:] * (1/Z[i])

The gram-matrix restructure avoids separate q/k projections (saves ~2.2
GFLOP/core) and all weight transposes except Wv's.
"""
import os
from contextlib import ExitStack

import numpy as np

_CACHE = {}

B, S, H, P = 8, 2048, 1024, 128


def _build():
    import concourse.tile as tile
    from concourse import bacc, mybir
    from concourse.masks import make_identity

    F32 = mybir.dt.float32
    F32R = mybir.dt.float32r
    I32 = mybir.dt.int32
    EXP = mybir.ActivationFunctionType.Exp

    ST = S // P
    HT = H // P
    IB = 256                # queries per i-block
    NIB = S // IB
    ISUB = IB // P
    NCH = H // 512

    nc = bacc.Bacc("TRN2", target_bir_lowering=False, debug=False,
                   num_devices=8)

    Xq = nc.dram_tensor("query", [S, H], F32, kind="ExternalInput").ap()
    Xk = nc.dram_tensor("key", [S, H], F32, kind="ExternalInput").ap()
    Xv = nc.dram_tensor("value", [S, H], F32, kind="ExternalInput").ap()
    Wq = nc.dram_tensor("Wq", [H, H], F32, kind="ExternalInput").ap()
    Wk = nc.dram_tensor("Wk", [H, H], F32, kind="ExternalInput").ap()
    Wv = nc.dram_tensor("Wv", [H, H], F32, kind="ExternalInput").ap()
    Msk = nc.dram_tensor("mask", [ST, P], I32, kind="ExternalInput").ap()
    Out = nc.dram_tensor("out", [S, H], F32, kind="ExternalOutput").ap()

    with tile.TileContext(nc) as tc, ExitStack() as root:
        const = root.enter_context(tc.tile_pool(name="const", bufs=1))

        ident = const.tile([P, P], F32, tag="id")
        make_identity(nc, ident[:])
        ones_f = const.tile([P, 4], F32, tag="onesf")
        nc.vector.memset(ones_f[:], 1.0)
        ones_r = const.tile([P, 4], F32R, tag="ones")
        nc.vector.tensor_copy(ones_r[:], ones_f[:])

        # mask -> per-key bias columns [128, ST]
        bias_cols = const.tile([P, ST], F32, tag="bias")
        with (
            tc.tile_pool(name="mtmp", bufs=1) as mtmp,
            tc.tile_pool(name="mps", bufs=1, space="PSUM") as mps,
        ):
            m_i = mtmp.tile([ST, P], I32, tag="mi")
            nc.sync.dma_start(m_i[:], Msk[:])
            m_f = mtmp.tile([ST, P], F32, tag="mf")
            nc.vector.tensor_copy(m_f[:], m_i[:])
            ps_m = mps.tile([P, ST], F32, tag="mt")
            nc.tensor.transpose(ps_m[:], m_f[:], ident[0:ST, 0:ST])
            nc.vector.tensor_scalar(
                bias_cols[:], ps_m[:], 50000.0, -50000.0,
                mybir.AluOpType.mult, mybir.AluOpType.add)

        c_pool = root.enter_context(tc.tile_pool(name="c", bufs=1))

        # phase 1a: G_kq[b,a] = sum_o Wk[o,b] Wq[o,a]
        m_tiles = []
        with ExitStack() as ph1a:
            mk_pool = ph1a.enter_context(tc.tile_pool(name="mkq", bufs=1))
            with ExitStack() as s1a:
                w_pool = s1a.enter_context(tc.tile_pool(name="wkq", bufs=1))
                ps1 = s1a.enter_context(tc.tile_pool(name="ps1a", bufs=4, space="PSUM"))
                wk_t, wq_t = [], []
                for o in range(HT):
                    tk = w_pool.tile([P, H], F32R, name=f"wk{o}", tag=f"wk{o}")
                    nc.gpsimd.dma_start(tk[:], Wk[o * P:(o + 1) * P, :])
                    wk_t.append(tk)
                    tq = w_pool.tile([P, H], F32R, name=f"wq{o}", tag=f"wq{o}")
                    nc.gpsimd.dma_start(tq[:], Wq[o * P:(o + 1) * P, :])
                    wq_t.append(tq)
                for b in range(HT):
                    mt = mk_pool.tile([P, H], F32R, name=f"m{b}", tag=f"m{b}")
                    m_tiles.append(mt)
                    for ach in range(NCH):
                        ps = ps1.tile([P, 512], F32, tag="mm")
                        for o in range(HT):
                            nc.tensor.matmul(
                                ps[:], wk_t[o][:, b * P:(b + 1) * P],
                                wq_t[o][:, ach * 512:(ach + 1) * 512],
                                start=(o == 0), stop=(o == HT - 1))
                        nc.vector.tensor_copy(mt[:, ach * 512:(ach + 1) * 512], ps[:])

            # phase 1b: C[a,j] = sum_b G[a,b] XkT[b,j]
            c_tiles = [c_pool.tile([P, S], F32R, name=f"c{a}", tag=f"c{a}")
                       for a in range(HT)]
            with ExitStack() as ph1b:
                xs_pool = ph1b.enter_context(tc.tile_pool(name="xk", bufs=2))
                xkt_pool = ph1b.enter_context(tc.tile_pool(name="xkt", bufs=2))
                pst = ph1b.enter_context(tc.tile_pool(name="ps1bt", bufs=2, space="PSUM"))
                psc = ph1b.enter_context(tc.tile_pool(name="ps1bc", bufs=4, space="PSUM"))
                JB = 512
                for jb in range(S // JB):
                    xkt = [xkt_pool.tile([P, JB], F32R, name=f"xkt{h}", tag=f"xkt{h}")
                           for h in range(HT)]
                    for jt in range(JB // P):
                        xn = xs_pool.tile([P, H], F32, tag="xn")
                        nc.sync.dma_start(
                            xn[:], Xk[jb * JB + jt * P:jb * JB + (jt + 1) * P, :])
                        for h in range(HT):
                            pt = pst.tile([P, P], F32, tag="t")
                            nc.tensor.transpose(pt[:], xn[:, h * P:(h + 1) * P], ident[:])
                            nc.vector.tensor_copy(xkt[h][:, jt * P:(jt + 1) * P], pt[:])
                    for a in range(HT):
                        ps = psc.tile([P, JB], F32, tag="mm")
                        for b in range(HT):
                            nc.tensor.matmul(
                                ps[:], m_tiles[b][:, a * P:(a + 1) * P], xkt[b][:],
                                start=(b == 0), stop=(b == HT - 1))
                        nc.vector.tensor_copy(c_tiles[a][:, jb * JB:(jb + 1) * JB], ps[:])

        # phase 1c: v[j,:] = sum_h XvT[h,j].T @ WvT[h,:]
        v_pool = root.enter_context(tc.tile_pool(name="v", bufs=1))
        v_tiles = [v_pool.tile([P, H], F32R, name=f"v{j}", tag=f"v{j}")
                   for j in range(ST)]
        with ExitStack() as ph1c:
            wv_pool = ph1c.enter_context(tc.tile_pool(name="wvT", bufs=1))
            wn_pool = ph1c.enter_context(tc.tile_pool(name="wvn", bufs=2))
            xs_pool = ph1c.enter_context(tc.tile_pool(name="xv", bufs=2))
            xvt_pool = ph1c.enter_context(tc.tile_pool(name="xvt", bufs=2))
            pst = ph1c.enter_context(tc.tile_pool(name="ps1ct", bufs=2, space="PSUM"))
            psv = ph1c.enter_context(tc.tile_pool(name="ps1cv", bufs=2, space="PSUM"))
            wvT = [wv_pool.tile([P, H], F32R, name=f"wvT{h}", tag=f"wvT{h}")
                   for h in range(HT)]
            for ot in range(HT):
                wn = wn_pool.tile([P, H], F32, tag="wn")
                nc.sync.dma_start(wn[:], Wv[ot * P:(ot + 1) * P, :])
                for h in range(HT):
                    pt = pst.tile([P, P], F32, tag="t")
                    nc.tensor.transpose(pt[:], wn[:, h * P:(h + 1) * P], ident[:])
                    nc.vector.tensor_copy(wvT[h][:, ot * P:(ot + 1) * P], pt[:])
            for j in range(ST):
                xn = xs_pool.tile([P, H], F32, tag="xn")
                nc.sync.dma_start(xn[:], Xv[j * P:(j + 1) * P, :])
                xvt = [xvt_pool.tile([P, P], F32R, name=f"xvt{h}", tag=f"xvt{h}")
                       for h in range(HT)]
                for h in range(HT):
                    pt = pst.tile([P, P], F32, tag="t")
                    nc.tensor.transpose(pt[:], xn[:, h * P:(h + 1) * P], ident[:])
                    nc.vector.tensor_copy(xvt[h][:], pt[:])
                vps = [psv.tile([P, 512], F32, name=f"vps{ch}", tag=f"mm{ch}")
                       for ch in range(NCH)]
                for h in range(HT):
                    for ch in range(NCH):
                        nc.tensor.matmul(
                            vps[ch][:], xvt[h][:], wvT[h][:, ch * 512:(ch + 1) * 512],
                            start=(h == 0), stop=(h == HT - 1))
                for ch in range(NCH):
                    nc.vector.tensor_copy(v_tiles[j][:, ch * 512:(ch + 1) * 512], vps[ch][:])

        # phase 2: attention per i-block
        with ExitStack() as ph2:
            xs_pool = ph2.enter_context(tc.tile_pool(name="xq", bufs=2))
            xqt_pool = ph2.enter_context(tc.tile_pool(name="xqt", bufs=2))
            e_pool = ph2.enter_context(tc.tile_pool(name="expT", bufs=4))
            o_pool = ph2.enter_context(tc.tile_pool(name="ctxo", bufs=4))
            rec_pool = ph2.enter_context(tc.tile_pool(name="rec", bufs=2))
            ps_tr = ph2.enter_context(tc.tile_pool(name="ps2t", bufs=1, space="PSUM"))
            ps_sc = ph2.enter_context(tc.tile_pool(name="ps2s", bufs=2, space="PSUM"))
            ps_ctx = ph2.enter_context(tc.tile_pool(name="ps2c", bufs=1, space="PSUM"))
            ps_z = ph2.enter_context(tc.tile_pool(name="ps2z", bufs=1, space="PSUM"))

            for ib in range(NIB):
                xqt = [xqt_pool.tile([P, IB], F32R, name=f"xqt{h}", tag=f"xqt{h}")
                       for h in range(HT)]
                for it in range(ISUB):
                    xn = xs_pool.tile([P, H], F32, tag="xn")
                    nc.sync.dma_start(
                        xn[:], Xq[ib * IB + it * P:ib * IB + (it + 1) * P, :])
                    for h in range(HT):
                        pt = ps_tr.tile([P, P], F32, tag="t")
                        nc.tensor.transpose(pt[:], xn[:, h * P:(h + 1) * P], ident[:])
                        nc.vector.tensor_copy(xqt[h][:, it * P:(it + 1) * P], pt[:])

                ctx_ps = [ps_ctx.tile([P, H], F32, name=f"ctx{i}", tag=f"ctx{i}")
                          for i in range(ISUB)]
                z_ps = ps_z.tile([P, 8], F32, tag="z")

                for jt in range(ST):
                    ps = ps_sc.tile([P, IB], F32, tag="sc")
                    for a in range(HT):
                        nc.tensor.matmul(
                            ps[:], c_tiles[a][:, jt * P:(jt + 1) * P], xqt[a][:],
                            start=(a == 0), stop=(a == HT - 1))
                    et = e_pool.tile([P, IB], F32R, tag="e")
                    nc.scalar.activation(et[:], ps[:], EXP,
                                         bias=bias_cols[:, jt:jt + 1], scale=1.0)
                    for isub in range(ISUB):
                        lhs = et[:, isub * P:(isub + 1) * P]
                        for ch in range(NCH):
                            nc.tensor.matmul(
                                ctx_ps[isub][:, ch * 512:(ch + 1) * 512],
                                lhs, v_tiles[jt][:, ch * 512:(ch + 1) * 512],
                                start=(jt == 0), stop=(jt == ST - 1))
                        nc.tensor.matmul(
                            z_ps[:, isub * 4:(isub + 1) * 4], lhs, ones_r[:],
                            start=(jt == 0 and isub == 0),
                            stop=(jt == ST - 1 and isub == ISUB - 1))

                rec = rec_pool.tile([P, ISUB], F32, tag="rec")
                nc.vector.reciprocal(rec[:], z_ps[:, 0:4 * ISUB:4])
                for isub in range(ISUB):
                    ot = o_pool.tile([P, H], F32, tag="o")
                    nc.vector.tensor_scalar_mul(ot[:], ctx_ps[isub][:], rec[:, isub:isub + 1])
                    nc.sync.dma_start(
                        Out[ib * IB + isub * P:ib * IB + (isub + 1) * P, :], ot[:])

    nc.compile()
    return nc


class _Runner:
    """Persistent PJRT executor mirroring bass2jax.run_bass_via_pjrt, built
    once so repeat kernel() calls skip jax retracing."""

    def __init__(self, nc, n_cores):
        import jax
        from jax.sharding import Mesh, PartitionSpec, NamedSharding
        from jax.experimental.shard_map import shard_map
        import concourse.mybir as mybir
        from concourse import bass2jax
        from concourse.bass2jax import _bass_exec_p, install_neuronx_cc_hook

        install_neuronx_cc_hook()
        self.jax = jax
        self.n_cores = n_cores
        partition_name = (nc.partition_id_tensor.name
                          if nc.partition_id_tensor else None)
        in_names, out_names, out_avals = [], [], []
        for alloc in nc.m.functions[0].allocations:
            if not isinstance(alloc, mybir.MemoryLocationSet):
                continue
            name = alloc.memorylocations[0].name
            if alloc.kind == "ExternalInput":
                if name != partition_name:
                    in_names.append(name)
            elif alloc.kind == "ExternalOutput":
                out_names.append(name)
                out_avals.append(jax.core.ShapedArray(
                    tuple(alloc.tensor_shape), mybir.dt.np(alloc.dtype)))
        self.in_names, self.out_names, self.out_avals = in_names, out_names, out_avals
        n_params, n_outs = len(in_names), len(out_avals)
        self.n_params = n_params
        all_names = list(in_names) + list(out_names)
        if partition_name is not None:
            all_names.append(partition_name)

        def _body(*args):
            operands = list(args)
            if partition_name is not None:
                operands.append(bass2jax.partition_id_tensor())
            return tuple(_bass_exec_p.bind(
                *operands,
                out_avals=tuple(out_avals),
                in_names=tuple(all_names),
                out_names=tuple(out_names),
                lowering_input_output_aliases=(),
                sim_require_finite=True,
                sim_require_nnan=True,
                nc=nc,
            ))

        devices = jax.devices()[:n_cores]
        assert len(devices) == n_cores, f"need {n_cores} neuron cores"
        mesh = Mesh(np.asarray(devices), ("core",))
        in_specs = (PartitionSpec("core"),) * (n_params + n_outs)
        out_specs = (PartitionSpec("core"),) * n_outs
        donate = tuple(range(n_params, n_params + n_outs))
        self._fn = jax.jit(
            shard_map(_body, mesh=mesh, in_specs=in_specs,
                      out_specs=out_specs, check_rep=False),
            donate_argnums=donate, keep_unused=True)
        self.sharding = NamedSharding(mesh, PartitionSpec("core"))

    def run(self, in_maps):
        jax = self.jax
        in_arrs = [
            jax.device_put(
                np.concatenate([np.ascontiguousarray(m[n]) for m in in_maps], axis=0),
                self.sharding)
            for n in self.in_names
        ]
        zeros = [
            jax.device_put(
                np.zeros((self.n_cores * a.shape[0], *a.shape[1:]), a.dtype),
                self.sharding)
            for a in self.out_avals
        ]
        outs = self._fn(*in_arrs, *zeros)
        res = []
        for c in range(self.n_cores):
            res.append({
                n: np.asarray(outs[i]).reshape(self.n_cores, *self.out_avals[i].shape)[c]
                for i, n in enumerate(self.out_names)})
        return res


def _get_runner():
    if "runner" not in _CACHE:
        nc = _build()
        _CACHE["runner"] = _Runner(nc, 8)
    return _CACHE["runner"]


def kernel(query, key, value, Wq, Wk, Wv, mask):
    query = np.asarray(query, dtype=np.float32)
    key = np.asarray(key, dtype=np.float32)
    value = np.asarray(value, dtype=np.float32)
    Wq = np.asarray(Wq, dtype=np.float32)
    Wk = np.asarray(Wk, dtype=np.float32)
    Wv = np.asarray(Wv, dtype=np.float32)
    mask = np.asarray(mask, dtype=np.int32)

    r = _get_runner()
    in_maps = []
    for c in range(B):
        in_maps.append({
            "query": query[c], "key": key[c], "value": value[c],
            "Wq": Wq, "Wk": Wk, "Wv": Wv,
            "mask": mask[c].reshape(S // P, P),
        })
    res = r.run(in_maps)
    return np.stack([res[c]["out"] for c in range(B)])



# revision 18
# speedup vs baseline: 175.0347x; 175.0347x over previous
"""Trainium2 Bass kernel for nn_Attention_43628277793473.

Single-head attention, B=8, S=2048, H=1024:
  q = query @ Wq.T ; k = key @ Wk.T ; v = value @ Wv.T
  score = q @ k.T ; masked_fill(mask==0, -99999) ; softmax ; out = attn @ v

Sharding: data-parallel over the batch dim - one batch element per
NeuronCore (8 cores), no collectives.

Host-side key compaction: masked keys contribute exactly zero
(exp(score - 50000) underflows to 0.0 in f32, matching the reference
where exp(-99999 - rowmax) underflows), so kernel() gathers only the
unmasked key/value rows per batch, pads to a multiple of 128, and sends
a per-key bias column (0 for real keys, -50000 for padding). With the
~50% random mask this halves the score/context/projection work.

Per-core algorithm (scores in float32r, context path in bfloat16):
  G[b,a]   = sum_o Wk[o,b] Wq[o,a]       # weight gram, natural layouts
  xqt      = Xq^T tiles (PE transpose, f32r) for all 2048 queries
  C[a,j]   = sum_b G[b,a] XkT[b,j]       # keys compacted to KT*128
  v[j,:]   = XvT.T @ WvT                 # bf16 operands, f32 PSUM
  scoreT   = C-tile.T @ xqt-slice        # keys on partitions
  expT     = exp(scoreT + bias_col)      # -> bf16
  ctx[i,:] += expT[:,i].T @ v_j          # bf16 matmuls, f32 PSUM
  Z[i]     += expT[:,i].T @ ones_bf16
  out[i,:] = ctx[i,:] * (1/Z[i])
"""
import os
from contextlib import ExitStack

import numpy as np

_CACHE = {}

B, S, H, P = 8, 2048, 1024, 128


def _build(KT):
    import concourse.tile as tile
    from concourse import bacc, mybir
    from concourse.masks import make_identity

    F32 = mybir.dt.float32
    F32R = mybir.dt.float32r
    BF16 = mybir.dt.bfloat16
    EXP = mybir.ActivationFunctionType.Exp

    HT = H // P              # 8 hidden tiles
    Spad = KT * P            # compacted+padded key count
    IB = 256                 # queries per i-block
    NIB = S // IB
    ISUB = IB // P
    NCH = H // 512

    nc = bacc.Bacc("TRN2", target_bir_lowering=False, debug=False,
                   num_devices=8)

    Xq = nc.dram_tensor("query", [S, H], F32, kind="ExternalInput").ap()
    Xk = nc.dram_tensor("key", [Spad, H], F32, kind="ExternalInput").ap()
    Xv = nc.dram_tensor("value", [Spad, H], F32, kind="ExternalInput").ap()
    Wq = nc.dram_tensor("Wq", [H, H], F32, kind="ExternalInput").ap()
    Wk = nc.dram_tensor("Wk", [H, H], F32, kind="ExternalInput").ap()
    Wv = nc.dram_tensor("Wv", [H, H], F32, kind="ExternalInput").ap()
    Bias = nc.dram_tensor("bias", [P, KT], F32, kind="ExternalInput").ap()
    Out = nc.dram_tensor("out", [S, H], F32, kind="ExternalOutput").ap()

    with tile.TileContext(nc) as tc, ExitStack() as root:
        const = root.enter_context(tc.tile_pool(name="const", bufs=1))

        ident_f = const.tile([P, P], F32, tag="idf")
        make_identity(nc, ident_f[:])
        ident_r = const.tile([P, P], F32R, tag="idr")
        nc.vector.tensor_copy(ident_r[:], ident_f[:])
        ident_b = const.tile([P, P], BF16, tag="idb")
        nc.vector.tensor_copy(ident_b[:], ident_f[:])
        ones_f = const.tile([P, 4], F32, tag="onesf")
        nc.vector.memset(ones_f[:], 1.0)
        ones_b = const.tile([P, 4], BF16, tag="ones")
        nc.vector.tensor_copy(ones_b[:], ones_f[:])
        bias_cols = const.tile([P, KT], F32, tag="bias")
        nc.sync.dma_start(bias_cols[:], Bias[:])

        # long-lived tensors
        c_pool = root.enter_context(tc.tile_pool(name="c", bufs=1))
        v_pool = root.enter_context(tc.tile_pool(name="v", bufs=1))

        with ExitStack() as ph1:
            mk_pool = ph1.enter_context(tc.tile_pool(name="mkq", bufs=1))
            m_tiles = []
            # phase 1a: G_kq[b,a] = sum_o Wk[o,b] Wq[o,a], plus Xq transposes
            with ExitStack() as s1a:
                w_pool = s1a.enter_context(tc.tile_pool(name="wkq", bufs=1))
                wqh_pool = s1a.enter_context(tc.tile_pool(name="wqh", bufs=1))
                ps1 = s1a.enter_context(tc.tile_pool(name="ps1a", bufs=4, space="PSUM"))
                wk_t = []
                for o in range(HT):
                    tk = w_pool.tile([P, H], F32R, name=f"wk{o}", tag=f"wk{o}")
                    nc.gpsimd.dma_start(tk[:], Wk[o * P:(o + 1) * P, :])
                    wk_t.append(tk)
                for b in range(HT):
                    mt = mk_pool.tile([P, H], F32R, name=f"m{b}", tag=f"m{b}")
                    m_tiles.append(mt)
                # G in two column-halves so only half of Wq is resident
                for ach in range(NCH):
                    wq_t = []
                    for o in range(HT):
                        tq = wqh_pool.tile([P, 512], F32R, name=f"wq{ach}_{o}",
                                           tag=f"wq{o}")
                        nc.gpsimd.dma_start(
                            tq[:], Wq[o * P:(o + 1) * P, ach * 512:(ach + 1) * 512])
                        wq_t.append(tq)
                    for b in range(HT):
                        ps = ps1.tile([P, 512], F32, tag="mm")
                        for o in range(HT):
                            nc.tensor.matmul(
                                ps[:], wk_t[o][:, b * P:(b + 1) * P], wq_t[o][:],
                                start=(o == 0), stop=(o == HT - 1))
                        nc.vector.tensor_copy(
                            m_tiles[b][:, ach * 512:(ach + 1) * 512], ps[:])

            # phase 1b: C[a,j] = sum_b G[b,a] XkT[b,j] over compacted keys,
            # interleaved with the Xq transposes (xqt resident for phase 2).
            # xqt_pool is entered on the root stack HERE (after the Wq/Wk
            # pool exits) so its 64KB never coexists with the weights.
            c_tiles = [c_pool.tile([P, Spad], F32R, name=f"c{a}", tag=f"c{a}")
                       for a in range(HT)]
            xqt = [c_pool.tile([P, S], F32R, name=f"xqt{h}", tag=f"xqt{h}")
                   for h in range(HT)]
            with ExitStack() as ph1b:
                xs_pool = ph1b.enter_context(tc.tile_pool(name="xk", bufs=3))
                xkt_pool = ph1b.enter_context(tc.tile_pool(name="xkt", bufs=1))
                pst = ph1b.enter_context(tc.tile_pool(name="ps1bt", bufs=4, space="PSUM"))
                psc = ph1b.enter_context(tc.tile_pool(name="ps1bc", bufs=2, space="PSUM"))
                jbs = []
                j0 = 0
                while j0 < KT:
                    nt = min(4, KT - j0)
                    jbs.append((j0, nt))
                    j0 += nt
                NQB = S // P          # 16 query row-blocks to transpose
                qb_next = 0

                def xq_transpose_block(it):
                    xn = xs_pool.tile([P, H], F32R, tag="xn")
                    nc.gpsimd.dma_start(xn[:], Xq[it * P:(it + 1) * P, :])
                    for h in range(HT):
                        pt = pst.tile([P, P], F32R, tag="t")
                        nc.tensor.transpose(pt[:], xn[:, h * P:(h + 1) * P], ident_r[:])
                        nc.vector.tensor_copy(xqt[h][:, it * P:(it + 1) * P], pt[:])

                for ci, (j0, nt) in enumerate(jbs):
                    JBw = nt * P
                    xkt = [xkt_pool.tile([P, JBw], F32R, name=f"xkt{h}", tag=f"xkt{h}")
                           for h in range(HT)]
                    for jt in range(nt):
                        xn = xs_pool.tile([P, H], F32R, tag="xn")
                        nc.gpsimd.dma_start(
                            xn[:], Xk[(j0 + jt) * P:(j0 + jt + 1) * P, :])
                        for h in range(HT):
                            pt = pst.tile([P, P], F32R, tag="t")
                            nc.tensor.transpose(pt[:], xn[:, h * P:(h + 1) * P], ident_r[:])
                            nc.vector.tensor_copy(xkt[h][:, jt * P:(jt + 1) * P], pt[:])
                    for a in range(HT):
                        ps = psc.tile([P, JBw], F32, tag="mm")
                        for b in range(HT):
                            nc.tensor.matmul(
                                ps[:], m_tiles[b][:, a * P:(a + 1) * P], xkt[b][:],
                                start=(b == 0), stop=(b == HT - 1))
                        nc.vector.tensor_copy(c_tiles[a][:, j0 * P:j0 * P + JBw], ps[:])
                    # spread the 16 Xq transpose blocks across the jb chunks
                    n_after = (NQB * (ci + 1)) // len(jbs)
                    while qb_next < n_after:
                        xq_transpose_block(qb_next)
                        qb_next += 1

        # phase 1c: v[j,:] = sum_h XvT[h,j].T @ WvT[h,:]  (bf16 operands)
        v_tiles = [v_pool.tile([P, H], BF16, name=f"v{j}", tag=f"v{j}")
                   for j in range(KT)]
        with ExitStack() as ph1c:
            wv_pool = ph1c.enter_context(tc.tile_pool(name="wvT", bufs=1))
            wn_pool = ph1c.enter_context(tc.tile_pool(name="wvn", bufs=2))
            wb_pool = ph1c.enter_context(tc.tile_pool(name="wvb", bufs=2))
            xs_pool = ph1c.enter_context(tc.tile_pool(name="xv", bufs=3))
            xb_pool = ph1c.enter_context(tc.tile_pool(name="xvb", bufs=2))
            xvt_pool = ph1c.enter_context(tc.tile_pool(name="xvt", bufs=2))
            pst = ph1c.enter_context(tc.tile_pool(name="ps1ct", bufs=2, space="PSUM"))
            psv = ph1c.enter_context(tc.tile_pool(name="ps1cv", bufs=2, space="PSUM"))
            wvT = [wv_pool.tile([P, H], BF16, name=f"wvT{h}", tag=f"wvT{h}")
                   for h in range(HT)]
            for ot in range(HT):
                wn = wn_pool.tile([P, H], F32, tag="wn")
                nc.gpsimd.dma_start(wn[:], Wv[ot * P:(ot + 1) * P, :])
                wb = wb_pool.tile([P, H], BF16, tag="wb")
                nc.vector.tensor_copy(wb[:], wn[:])
                for h in range(HT):
                    pt = pst.tile([P, P], BF16, tag="t")
                    nc.tensor.transpose(pt[:], wb[:, h * P:(h + 1) * P], ident_b[:])
                    nc.vector.tensor_copy(wvT[h][:, ot * P:(ot + 1) * P], pt[:])
            for kt in range(KT):
                xn = xs_pool.tile([P, H], F32, tag="xn")
                nc.sync.dma_start(xn[:], Xv[kt * P:(kt + 1) * P, :])
                xb = xb_pool.tile([P, H], BF16, tag="xb")
                nc.vector.tensor_copy(xb[:], xn[:])
                xvt = [xvt_pool.tile([P, P], BF16, name=f"xvt{h}", tag=f"xvt{h}")
                       for h in range(HT)]
                for h in range(HT):
                    pt = pst.tile([P, P], BF16, tag="t")
                    nc.tensor.transpose(pt[:], xb[:, h * P:(h + 1) * P], ident_b[:])
                    nc.vector.tensor_copy(xvt[h][:], pt[:])
                vps = [psv.tile([P, 512], F32, name=f"vps{ch}", tag=f"mm{ch}")
                       for ch in range(NCH)]
                for h in range(HT):
                    for ch in range(NCH):
                        nc.tensor.matmul(
                            vps[ch][:], xvt[h][:], wvT[h][:, ch * 512:(ch + 1) * 512],
                            start=(h == 0), stop=(h == HT - 1))
                for ch in range(NCH):
                    nc.vector.tensor_copy(v_tiles[kt][:, ch * 512:(ch + 1) * 512], vps[ch][:])

        # phase 2: attention per i-block
        with ExitStack() as ph2:
            e_pool = ph2.enter_context(tc.tile_pool(name="expT", bufs=4))
            o_pool = ph2.enter_context(tc.tile_pool(name="ctxo", bufs=4))
            rec_pool = ph2.enter_context(tc.tile_pool(name="rec", bufs=2))
            ps_sc = ph2.enter_context(tc.tile_pool(name="ps2s", bufs=2, space="PSUM"))
            ps_ctx = ph2.enter_context(tc.tile_pool(name="ps2c", bufs=1, space="PSUM"))
            ps_z = ph2.enter_context(tc.tile_pool(name="ps2z", bufs=1, space="PSUM"))

            for ib in range(NIB):
                ctx_ps = [ps_ctx.tile([P, H], F32, name=f"ctx{i}", tag=f"ctx{i}")
                          for i in range(ISUB)]
                z_ps = ps_z.tile([P, 8], F32, tag="z")

                for jt in range(KT):
                    ps = ps_sc.tile([P, IB], F32, tag="sc")
                    for a in range(HT):
                        nc.tensor.matmul(
                            ps[:], c_tiles[a][:, jt * P:(jt + 1) * P],
                            xqt[a][:, ib * IB:(ib + 1) * IB],
                            start=(a == 0), stop=(a == HT - 1))
                    et = e_pool.tile([P, IB], BF16, tag="e")
                    nc.scalar.activation(et[:], ps[:], EXP,
                                         bias=bias_cols[:, jt:jt + 1], scale=1.0)
                    for isub in range(ISUB):
                        lhs = et[:, isub * P:(isub + 1) * P]
                        for ch in range(NCH):
                            nc.tensor.matmul(
                                ctx_ps[isub][:, ch * 512:(ch + 1) * 512],
                                lhs, v_tiles[jt][:, ch * 512:(ch + 1) * 512],
                                start=(jt == 0), stop=(jt == KT - 1))
                        nc.tensor.matmul(
                            z_ps[:, isub * 4:(isub + 1) * 4], lhs, ones_b[:],
                            start=(jt == 0 and isub == 0),
                            stop=(jt == KT - 1 and isub == ISUB - 1))

                rec = rec_pool.tile([P, ISUB], F32, tag="rec")
                nc.vector.reciprocal(rec[:], z_ps[:, 0:4 * ISUB:4])
                for isub in range(ISUB):
                    ot = o_pool.tile([P, H], F32, tag="o")
                    nc.vector.tensor_scalar_mul(ot[:], ctx_ps[isub][:], rec[:, isub:isub + 1])
                    nc.sync.dma_start(
                        Out[ib * IB + isub * P:ib * IB + (isub + 1) * P, :], ot[:])

    nc.compile()
    return nc


class _Runner:
    """Persistent PJRT executor mirroring bass2jax.run_bass_via_pjrt, built
    once so repeat kernel() calls skip jax retracing."""

    def __init__(self, nc, n_cores):
        import jax
        from jax.sharding import Mesh, PartitionSpec, NamedSharding
        from jax.experimental.shard_map import shard_map
        import concourse.mybir as mybir
        from concourse import bass2jax
        from concourse.bass2jax import _bass_exec_p, install_neuronx_cc_hook

        install_neuronx_cc_hook()
        self.jax = jax
        self.nc = nc
        self.n_cores = n_cores
        partition_name = (nc.partition_id_tensor.name
                          if nc.partition_id_tensor else None)
        in_names, out_names, out_avals = [], [], []
        for alloc in nc.m.functions[0].allocations:
            if not isinstance(alloc, mybir.MemoryLocationSet):
                continue
            name = alloc.memorylocations[0].name
            if alloc.kind == "ExternalInput":
                if name != partition_name:
                    in_names.append(name)
            elif alloc.kind == "ExternalOutput":
                out_names.append(name)
                out_avals.append(jax.core.ShapedArray(
                    tuple(alloc.tensor_shape), mybir.dt.np(alloc.dtype)))
        self.in_names, self.out_names, self.out_avals = in_names, out_names, out_avals
        n_params, n_outs = len(in_names), len(out_avals)
        self.n_params = n_params
        all_names = list(in_names) + list(out_names)
        if partition_name is not None:
            all_names.append(partition_name)

        def _body(*args):
            operands = list(args)
            if partition_name is not None:
                operands.append(bass2jax.partition_id_tensor())
            return tuple(_bass_exec_p.bind(
                *operands,
                out_avals=tuple(out_avals),
                in_names=tuple(all_names),
                out_names=tuple(out_names),
                lowering_input_output_aliases=(),
                sim_require_finite=True,
                sim_require_nnan=True,
                nc=nc,
            ))

        devices = jax.devices()[:n_cores]
        assert len(devices) == n_cores, f"need {n_cores} neuron cores"
        mesh = Mesh(np.asarray(devices), ("core",))
        in_specs = (PartitionSpec("core"),) * (n_params + n_outs)
        out_specs = (PartitionSpec("core"),) * n_outs
        donate = tuple(range(n_params, n_params + n_outs))
        self._fn = jax.jit(
            shard_map(_body, mesh=mesh, in_specs=in_specs,
                      out_specs=out_specs, check_rep=False),
            donate_argnums=donate, keep_unused=True)
        self.sharding = NamedSharding(mesh, PartitionSpec("core"))

    def run(self, in_maps):
        jax = self.jax
        in_arrs = [
            jax.device_put(
                np.concatenate([np.ascontiguousarray(m[n]) for m in in_maps], axis=0),
                self.sharding)
            for n in self.in_names
        ]
        zeros = [
            jax.device_put(
                np.zeros((self.n_cores * a.shape[0], *a.shape[1:]), a.dtype),
                self.sharding)
            for a in self.out_avals
        ]
        outs = self._fn(*in_arrs, *zeros)
        res = []
        for c in range(self.n_cores):
            res.append({
                n: np.asarray(outs[i]).reshape(self.n_cores, *self.out_avals[i].shape)[c]
                for i, n in enumerate(self.out_names)})
        return res


def _get_runner(KT):
    key = ("runner", KT)
    if key not in _CACHE:
        nc = _build(KT)
        _CACHE[key] = _Runner(nc, 8)
    return _CACHE[key]


def _make_in_maps(query, key, value, Wq, Wk, Wv, mask, KT, idxs):
    Spad = KT * P
    in_maps = []
    for c in range(B):
        idx = idxs[c]
        n = len(idx)
        kg = np.zeros((Spad, H), np.float32)
        vg = np.zeros((Spad, H), np.float32)
        if n:
            kg[:n] = key[c][idx]
            vg[:n] = value[c][idx]
        bias = np.full((Spad,), -50000.0, np.float32)
        bias[:n] = 0.0
        bias2d = np.ascontiguousarray(bias.reshape(KT, P).T)
        in_maps.append({
            "query": query[c], "key": kg, "value": vg,
            "Wq": Wq, "Wk": Wk, "Wv": Wv, "bias": bias2d,
        })
    return in_maps


def kernel(query, key, value, Wq, Wk, Wv, mask):
    query = np.asarray(query, dtype=np.float32)
    key = np.asarray(key, dtype=np.float32)
    value = np.asarray(value, dtype=np.float32)
    Wq = np.asarray(Wq, dtype=np.float32)
    Wk = np.asarray(Wk, dtype=np.float32)
    Wv = np.asarray(Wv, dtype=np.float32)
    mask = np.asarray(mask, dtype=np.int32)

    idxs = [np.flatnonzero(mask[c]) for c in range(B)]
    KT = max(1, (max(len(i) for i in idxs) + P - 1) // P)

    r = _get_runner(KT)
    in_maps = _make_in_maps(query, key, value, Wq, Wk, Wv, mask, KT, idxs)
    res = r.run(in_maps)
    out = np.stack([res[c]["out"] for c in range(B)])

    # a batch with every key masked: reference softmax is uniform over all
    # keys (all scores equal -99999), so ctx = mean(v) for every query row
    for c in range(B):
        if len(idxs[c]) == 0:
            v_mean = (value[c].mean(0) @ Wv.T).astype(np.float32)
            out[c][:] = v_mean[None, :]
    return out


# revision 23
# speedup vs baseline: 182.3818x; 1.0420x over previous
"""Trainium2 Bass kernel for nn_Attention_43628277793473.

Single-head attention, B=8, S=2048, H=1024:
  q = query @ Wq.T ; k = key @ Wk.T ; v = value @ Wv.T
  score = q @ k.T ; masked_fill(mask==0, -99999) ; softmax ; out = attn @ v

Sharding: data-parallel over the batch dim - one batch element per
NeuronCore (8 cores), no collectives.

Host-side key compaction: masked keys contribute exactly zero
(exp(score - 50000) underflows to 0.0 in f32, matching the reference
where exp(-99999 - rowmax) underflows), so kernel() gathers only the
unmasked key/value rows per batch, pads to a multiple of 128, and sends
a per-key bias column (0 for real keys, -50000 for padding). With the
~50% random mask this halves the score/context/projection work.

Per-core algorithm (scores in float32r, context path in bfloat16):
  G[b,a]   = sum_o Wk[o,b] Wq[o,a]       # weight gram, natural layouts
  xqt      = Xq^T tiles (PE transpose, f32r) for all 2048 queries
  C[a,j]   = sum_b G[b,a] XkT[b,j]       # keys compacted to KT*128
  v[j,:]   = XvT.T @ WvT                 # bf16 operands, f32 PSUM
  scoreT   = C-tile.T @ xqt-slice        # keys on partitions
  expT     = exp(scoreT + bias_col)      # -> bf16
  ctx[i,:] += expT[:,i].T @ v_j          # bf16 matmuls, f32 PSUM
  Z[i]     += expT[:,i].T @ ones_bf16
  out[i,:] = ctx[i,:] * (1/Z[i])
"""
import os
from contextlib import ExitStack

import numpy as np

_CACHE = {}

B, S, H, P = 8, 2048, 1024, 128


def _build(KT):
    import concourse.tile as tile
    from concourse import bacc, mybir
    from concourse.masks import make_identity

    F32 = mybir.dt.float32
    F32R = mybir.dt.float32r
    BF16 = mybir.dt.bfloat16
    EXP = mybir.ActivationFunctionType.Exp

    HT = H // P              # 8 hidden tiles
    Spad = KT * P            # compacted+padded key count
    IB = 256                 # queries per i-block
    NIB = S // IB
    ISUB = IB // P
    NCH = H // 512

    nc = bacc.Bacc("TRN2", target_bir_lowering=False, debug=False,
                   num_devices=8)

    Xq = nc.dram_tensor("query", [S, H], F32R, kind="ExternalInput").ap()
    Xk = nc.dram_tensor("key", [Spad, H], F32R, kind="ExternalInput").ap()
    Xv = nc.dram_tensor("value", [Spad, H], F32, kind="ExternalInput").ap()
    Wq = nc.dram_tensor("Wq", [H, H], F32, kind="ExternalInput").ap()
    Wk = nc.dram_tensor("Wk", [H, H], F32, kind="ExternalInput").ap()
    Wv = nc.dram_tensor("Wv", [H, H], F32, kind="ExternalInput").ap()
    Bias = nc.dram_tensor("bias", [P, KT], F32, kind="ExternalInput").ap()
    Out = nc.dram_tensor("out", [S, H], F32, kind="ExternalOutput").ap()

    with tile.TileContext(nc) as tc, ExitStack() as root:
        const = root.enter_context(tc.tile_pool(name="const", bufs=1))

        ident_f = const.tile([P, P], F32, tag="idf")
        make_identity(nc, ident_f[:])
        ident_r = const.tile([P, P], F32R, tag="idr")
        nc.vector.tensor_copy(ident_r[:], ident_f[:])
        ident_b = const.tile([P, P], BF16, tag="idb")
        nc.vector.tensor_copy(ident_b[:], ident_f[:])
        ones_f = const.tile([P, 4], F32, tag="onesf")
        nc.vector.memset(ones_f[:], 1.0)
        ones_b = const.tile([P, 4], BF16, tag="ones")
        nc.vector.tensor_copy(ones_b[:], ones_f[:])
        bias_cols = const.tile([P, KT], F32, tag="bias")
        nc.sync.dma_start(bias_cols[:], Bias[:])

        # long-lived tensors
        c_pool = root.enter_context(tc.tile_pool(name="c", bufs=1))
        v_pool = root.enter_context(tc.tile_pool(name="v", bufs=1))

        with ExitStack() as ph1:
            mk_pool = ph1.enter_context(tc.tile_pool(name="mkq", bufs=1))
            m_tiles = []
            # phase 1a: G_kq[b,a] = sum_o Wk[o,b] Wq[o,a], plus Xq transposes
            with ExitStack() as s1a:
                w_pool = s1a.enter_context(tc.tile_pool(name="wkq", bufs=1))
                wqh_pool = s1a.enter_context(tc.tile_pool(name="wqh", bufs=1))
                ps1 = s1a.enter_context(tc.tile_pool(name="ps1a", bufs=4, space="PSUM"))
                wk_t = []
                for o in range(HT):
                    tk = w_pool.tile([P, H], F32R, name=f"wk{o}", tag=f"wk{o}")
                    nc.gpsimd.dma_start(tk[:], Wk[o * P:(o + 1) * P, :])
                    wk_t.append(tk)
                for b in range(HT):
                    mt = mk_pool.tile([P, H], F32R, name=f"m{b}", tag=f"m{b}")
                    m_tiles.append(mt)
                # G in two column-halves so only half of Wq is resident
                for ach in range(NCH):
                    wq_t = []
                    for o in range(HT):
                        tq = wqh_pool.tile([P, 512], F32R, name=f"wq{ach}_{o}",
                                           tag=f"wq{o}")
                        nc.gpsimd.dma_start(
                            tq[:], Wq[o * P:(o + 1) * P, ach * 512:(ach + 1) * 512])
                        wq_t.append(tq)
                    for b in range(HT):
                        ps = ps1.tile([P, 512], F32, tag="mm")
                        for o in range(HT):
                            nc.tensor.matmul(
                                ps[:], wk_t[o][:, b * P:(b + 1) * P], wq_t[o][:],
                                start=(o == 0), stop=(o == HT - 1))
                        nc.vector.tensor_copy(
                            m_tiles[b][:, ach * 512:(ach + 1) * 512], ps[:])

            # phase 1b: C[a,j] = sum_b G[b,a] XkT[b,j] over compacted keys,
            # interleaved with the Xq transposes (xqt resident for phase 2).
            # xqt_pool is entered on the root stack HERE (after the Wq/Wk
            # pool exits) so its 64KB never coexists with the weights.
            c_tiles = [c_pool.tile([P, Spad], F32R, name=f"c{a}", tag=f"c{a}")
                       for a in range(HT)]
            xqt = [c_pool.tile([P, S], F32R, name=f"xqt{h}", tag=f"xqt{h}")
                   for h in range(HT)]
            with ExitStack() as ph1b:
                xs_pool = ph1b.enter_context(tc.tile_pool(name="xk", bufs=3))
                xkt_pool = ph1b.enter_context(tc.tile_pool(name="xkt", bufs=2))
                pst = ph1b.enter_context(tc.tile_pool(name="ps1bt", bufs=4, space="PSUM"))
                psc = ph1b.enter_context(tc.tile_pool(name="ps1bc", bufs=2, space="PSUM"))
                jbs = []
                j0 = 0
                while j0 < KT:
                    nt = min(4, KT - j0)
                    jbs.append((j0, nt))
                    j0 += nt
                NQB = S // P          # 16 query row-blocks to transpose
                qb_next = 0

                def xq_transpose_block(it):
                    xn = xs_pool.tile([P, H], F32R, tag="xn")
                    nc.sync.dma_start(xn[:], Xq[it * P:(it + 1) * P, :])
                    for h in range(HT):
                        pt = pst.tile([P, P], F32R, tag="t")
                        nc.tensor.transpose(pt[:], xn[:, h * P:(h + 1) * P], ident_r[:])
                        nc.vector.tensor_copy(xqt[h][:, it * P:(it + 1) * P], pt[:])

                for ci, (j0, nt) in enumerate(jbs):
                    JBw = nt * P
                    xkt = [xkt_pool.tile([P, JBw], F32R, name=f"xkt{h}", tag=f"xkt{h}")
                           for h in range(HT)]
                    for jt in range(nt):
                        xn = xs_pool.tile([P, H], F32R, tag="xn")
                        nc.sync.dma_start(
                            xn[:], Xk[(j0 + jt) * P:(j0 + jt + 1) * P, :])
                        for h in range(HT):
                            pt = pst.tile([P, P], F32R, tag="t")
                            nc.tensor.transpose(pt[:], xn[:, h * P:(h + 1) * P], ident_r[:])
                            nc.vector.tensor_copy(xkt[h][:, jt * P:(jt + 1) * P], pt[:])
                    for a in range(HT):
                        ps = psc.tile([P, JBw], F32, tag="mm")
                        for b in range(HT):
                            nc.tensor.matmul(
                                ps[:], m_tiles[b][:, a * P:(a + 1) * P], xkt[b][:],
                                start=(b == 0), stop=(b == HT - 1))
                        nc.vector.tensor_copy(c_tiles[a][:, j0 * P:j0 * P + JBw], ps[:])
                    # spread the 16 Xq transpose blocks across the jb chunks
                    n_after = (NQB * (ci + 1)) // len(jbs)
                    while qb_next < n_after:
                        xq_transpose_block(qb_next)
                        qb_next += 1

        # phase 1c: v[j,:] = sum_h XvT[h,j].T @ WvT[h,:]  (bf16 operands)
        v_tiles = [v_pool.tile([P, H], BF16, name=f"v{j}", tag=f"v{j}")
                   for j in range(KT)]
        with ExitStack() as ph1c:
            wv_pool = ph1c.enter_context(tc.tile_pool(name="wvT", bufs=1))
            wb_pool = ph1c.enter_context(tc.tile_pool(name="wvb", bufs=3))
            xb_pool = ph1c.enter_context(tc.tile_pool(name="xvb", bufs=3))
            xvt_pool = ph1c.enter_context(tc.tile_pool(name="xvt", bufs=1))
            pst = ph1c.enter_context(tc.tile_pool(name="ps1ct", bufs=2, space="PSUM"))
            psv = ph1c.enter_context(tc.tile_pool(name="ps1cv", bufs=2, space="PSUM"))
            wvT = [wv_pool.tile([P, H], BF16, name=f"wvT{h}", tag=f"wvT{h}")
                   for h in range(HT)]
            for ot in range(HT):
                wb = wb_pool.tile([P, H], BF16, tag="wb")
                nc.gpsimd.dma_start(wb[:], Wv[ot * P:(ot + 1) * P, :])
                for h in range(HT):
                    pt = pst.tile([P, P], BF16, tag="t")
                    nc.tensor.transpose(pt[:], wb[:, h * P:(h + 1) * P], ident_b[:])
                    nc.vector.tensor_copy(wvT[h][:, ot * P:(ot + 1) * P], pt[:])
            # all Xv transposes first (resident), then a dense v matmul stream
            xvt = [[xvt_pool.tile([P, P], BF16, name=f"xvt{kt}_{h}",
                                  tag=f"xvt{kt}_{h}") for h in range(HT)]
                   for kt in range(KT)]
            for kt in range(KT):
                xb = xb_pool.tile([P, H], BF16, tag="xb")
                nc.gpsimd.dma_start(xb[:], Xv[kt * P:(kt + 1) * P, :])
                for h in range(HT):
                    pt = pst.tile([P, P], BF16, tag="t")
                    nc.tensor.transpose(pt[:], xb[:, h * P:(h + 1) * P], ident_b[:])
                    nc.vector.tensor_copy(xvt[kt][h][:], pt[:])
            for kt in range(KT):
                vps = [psv.tile([P, 512], F32, name=f"vps{ch}", tag=f"mm{ch}")
                       for ch in range(NCH)]
                for h in range(HT):
                    for ch in range(NCH):
                        nc.tensor.matmul(
                            vps[ch][:], xvt[kt][h][:], wvT[h][:, ch * 512:(ch + 1) * 512],
                            start=(h == 0), stop=(h == HT - 1))
                for ch in range(NCH):
                    nc.vector.tensor_copy(v_tiles[kt][:, ch * 512:(ch + 1) * 512], vps[ch][:])

        # phase 2: attention per i-block
        with ExitStack() as ph2:
            e_pool = ph2.enter_context(tc.tile_pool(name="expT", bufs=4))
            o_pool = ph2.enter_context(tc.tile_pool(name="ctxo", bufs=4))
            rec_pool = ph2.enter_context(tc.tile_pool(name="rec", bufs=2))
            ps_sc = ph2.enter_context(tc.tile_pool(name="ps2s", bufs=2, space="PSUM"))
            ps_ctx = ph2.enter_context(tc.tile_pool(name="ps2c", bufs=1, space="PSUM"))
            ps_z = ph2.enter_context(tc.tile_pool(name="ps2z", bufs=1, space="PSUM"))

            for ib in range(NIB):
                ctx_ps = [ps_ctx.tile([P, H], F32, name=f"ctx{i}", tag=f"ctx{i}")
                          for i in range(ISUB)]
                z_ps = ps_z.tile([P, 8], F32, tag="z")

                for jt in range(KT):
                    ps = ps_sc.tile([P, IB], F32, tag="sc")
                    for a in range(HT):
                        nc.tensor.matmul(
                            ps[:], c_tiles[a][:, jt * P:(jt + 1) * P],
                            xqt[a][:, ib * IB:(ib + 1) * IB],
                            start=(a == 0), stop=(a == HT - 1))
                    et = e_pool.tile([P, IB], BF16, tag="e")
                    nc.scalar.activation(et[:], ps[:], EXP,
                                         bias=bias_cols[:, jt:jt + 1], scale=1.0)
                    for isub in range(ISUB):
                        lhs = et[:, isub * P:(isub + 1) * P]
                        for ch in range(NCH):
                            nc.tensor.matmul(
                                ctx_ps[isub][:, ch * 512:(ch + 1) * 512],
                                lhs, v_tiles[jt][:, ch * 512:(ch + 1) * 512],
                                start=(jt == 0), stop=(jt == KT - 1))
                        nc.tensor.matmul(
                            z_ps[:, isub * 4:(isub + 1) * 4], lhs, ones_b[:],
                            start=(jt == 0 and isub == 0),
                            stop=(jt == KT - 1 and isub == ISUB - 1))

                rec = rec_pool.tile([P, ISUB], F32, tag="rec")
                nc.vector.reciprocal(rec[:], z_ps[:, 0:4 * ISUB:4])
                for isub in range(ISUB):
                    ot = o_pool.tile([P, H], F32, tag="o")
                    nc.vector.tensor_scalar_mul(ot[:], ctx_ps[isub][:], rec[:, isub:isub + 1])
                    nc.sync.dma_start(
                        Out[ib * IB + isub * P:ib * IB + (isub + 1) * P, :], ot[:])

    nc.compile()
    return nc


class _Runner:
    """Persistent PJRT executor mirroring bass2jax.run_bass_via_pjrt, built
    once so repeat kernel() calls skip jax retracing."""

    def __init__(self, nc, n_cores):
        import jax
        from jax.sharding import Mesh, PartitionSpec, NamedSharding
        from jax.experimental.shard_map import shard_map
        import concourse.mybir as mybir
        from concourse import bass2jax
        from concourse.bass2jax import _bass_exec_p, install_neuronx_cc_hook

        install_neuronx_cc_hook()
        self.jax = jax
        self.nc = nc
        self.n_cores = n_cores
        partition_name = (nc.partition_id_tensor.name
                          if nc.partition_id_tensor else None)
        in_names, out_names, out_avals = [], [], []
        for alloc in nc.m.functions[0].allocations:
            if not isinstance(alloc, mybir.MemoryLocationSet):
                continue
            name = alloc.memorylocations[0].name
            if alloc.kind == "ExternalInput":
                if name != partition_name:
                    in_names.append(name)
            elif alloc.kind == "ExternalOutput":
                out_names.append(name)
                out_avals.append(jax.core.ShapedArray(
                    tuple(alloc.tensor_shape), mybir.dt.np(alloc.dtype)))
        self.in_names, self.out_names, self.out_avals = in_names, out_names, out_avals
        n_params, n_outs = len(in_names), len(out_avals)
        self.n_params = n_params
        all_names = list(in_names) + list(out_names)
        if partition_name is not None:
            all_names.append(partition_name)

        def _body(*args):
            operands = list(args)
            if partition_name is not None:
                operands.append(bass2jax.partition_id_tensor())
            return tuple(_bass_exec_p.bind(
                *operands,
                out_avals=tuple(out_avals),
                in_names=tuple(all_names),
                out_names=tuple(out_names),
                lowering_input_output_aliases=(),
                sim_require_finite=True,
                sim_require_nnan=True,
                nc=nc,
            ))

        devices = jax.devices()[:n_cores]
        assert len(devices) == n_cores, f"need {n_cores} neuron cores"
        mesh = Mesh(np.asarray(devices), ("core",))
        in_specs = (PartitionSpec("core"),) * (n_params + n_outs)
        out_specs = (PartitionSpec("core"),) * n_outs
        donate = tuple(range(n_params, n_params + n_outs))
        self._fn = jax.jit(
            shard_map(_body, mesh=mesh, in_specs=in_specs,
                      out_specs=out_specs, check_rep=False),
            donate_argnums=donate, keep_unused=True)
        self.sharding = NamedSharding(mesh, PartitionSpec("core"))

    def run(self, in_maps):
        jax = self.jax
        in_arrs = [
            jax.device_put(
                np.concatenate([np.ascontiguousarray(m[n]) for m in in_maps], axis=0),
                self.sharding)
            for n in self.in_names
        ]
        zeros = [
            jax.device_put(
                np.zeros((self.n_cores * a.shape[0], *a.shape[1:]), a.dtype),
                self.sharding)
            for a in self.out_avals
        ]
        outs = self._fn(*in_arrs, *zeros)
        res = []
        for c in range(self.n_cores):
            res.append({
                n: np.asarray(outs[i]).reshape(self.n_cores, *self.out_avals[i].shape)[c]
                for i, n in enumerate(self.out_names)})
        return res


def _get_runner(KT):
    key = ("runner", KT)
    if key not in _CACHE:
        nc = _build(KT)
        _CACHE[key] = _Runner(nc, 8)
    return _CACHE[key]


def _make_in_maps(query, key, value, Wq, Wk, Wv, mask, KT, idxs):
    Spad = KT * P
    in_maps = []
    for c in range(B):
        idx = idxs[c]
        n = len(idx)
        kg = np.zeros((Spad, H), np.float32)
        vg = np.zeros((Spad, H), np.float32)
        if n:
            kg[:n] = key[c][idx]
            vg[:n] = value[c][idx]
        bias = np.full((Spad,), -50000.0, np.float32)
        bias[:n] = 0.0
        bias2d = np.ascontiguousarray(bias.reshape(KT, P).T)
        in_maps.append({
            "query": query[c], "key": kg, "value": vg,
            "Wq": Wq, "Wk": Wk, "Wv": Wv, "bias": bias2d,
        })
    return in_maps


def kernel(query, key, value, Wq, Wk, Wv, mask):
    query = np.asarray(query, dtype=np.float32)
    key = np.asarray(key, dtype=np.float32)
    value = np.asarray(value, dtype=np.float32)
    Wq = np.asarray(Wq, dtype=np.float32)
    Wk = np.asarray(Wk, dtype=np.float32)
    Wv = np.asarray(Wv, dtype=np.float32)
    mask = np.asarray(mask, dtype=np.int32)

    idxs = [np.flatnonzero(mask[c]) for c in range(B)]
    KT = max(1, (max(len(i) for i in idxs) + P - 1) // P)

    r = _get_runner(KT)
    in_maps = _make_in_maps(query, key, value, Wq, Wk, Wv, mask, KT, idxs)
    res = r.run(in_maps)
    out = np.stack([res[c]["out"] for c in range(B)])

    # a batch with every key masked: reference softmax is uniform over all
    # keys (all scores equal -99999), so ctx = mean(v) for every query row
    for c in range(B):
        if len(idxs[c]) == 0:
            v_mean = (value[c].mean(0) @ Wv.T).astype(np.float32)
            out[c][:] = v_mean[None, :]
    return out


# revision 33
# speedup vs baseline: 187.3940x; 1.0275x over previous
"""Trainium2 Bass kernel for nn_Attention_43628277793473.

Single-head attention, B=8, S=2048, H=1024:
  q = query @ Wq.T ; k = key @ Wk.T ; v = value @ Wv.T
  score = q @ k.T ; masked_fill(mask==0, -99999) ; softmax ; out = attn @ v

Sharding: data-parallel over the batch dim - one batch element per
NeuronCore (8 cores), no collectives.

Host-side key compaction: masked keys contribute exactly zero
(exp(score - 50000) underflows to 0.0 in f32, matching the reference
where exp(-99999 - rowmax) underflows), so kernel() gathers only the
unmasked key/value rows per batch, pads to a multiple of 128, and sends
a per-key bias column (0 for real keys, -50000 for padding). With the
~50% random mask this halves the score/context/projection work.

Per-core algorithm (scores in float32r, context path in bfloat16):
  G[b,a]   = sum_o Wk[o,b] Wq[o,a]       # weight gram, natural layouts
  xqt      = Xq^T tiles (PE transpose, f32r) for all 2048 queries
  C[a,j]   = sum_b G[b,a] XkT[b,j]       # keys compacted to KT*128
  v[j,:]   = XvT.T @ WvT                 # bf16 operands, f32 PSUM
  scoreT   = C-tile.T @ xqt-slice        # keys on partitions
  expT     = exp(scoreT + bias_col)      # -> bf16
  ctx[i,:] += expT[:,i].T @ v_j          # bf16 matmuls, f32 PSUM
  Z[i]     += expT[:,i].T @ ones_bf16
  out[i,:] = ctx[i,:] * (1/Z[i])
"""
import os
from contextlib import ExitStack

import numpy as np

_CACHE = {}

B, S, H, P = 8, 2048, 1024, 128


def _build(KT):
    import concourse.tile as tile
    from concourse import bacc, mybir
    from concourse.masks import make_identity

    F32 = mybir.dt.float32
    F32R = mybir.dt.float32r
    BF16 = mybir.dt.bfloat16
    EXP = mybir.ActivationFunctionType.Exp

    HT = H // P              # 8 hidden tiles
    Spad = KT * P            # compacted+padded key count
    IB = 256                 # queries per i-block
    NIB = S // IB
    ISUB = IB // P
    NCH = H // 512

    nc = bacc.Bacc("TRN2", target_bir_lowering=False, debug=False,
                   num_devices=8)

    Xq = nc.dram_tensor("query", [S, H], F32R, kind="ExternalInput").ap()
    Xk = nc.dram_tensor("key", [Spad, H], F32R, kind="ExternalInput").ap()
    Xv = nc.dram_tensor("value", [Spad, H], F32, kind="ExternalInput").ap()
    Wq = nc.dram_tensor("Wq", [H, H], F32, kind="ExternalInput").ap()
    Wk = nc.dram_tensor("Wk", [H, H], F32, kind="ExternalInput").ap()
    Wv = nc.dram_tensor("Wv", [H, H], F32, kind="ExternalInput").ap()
    Bias = nc.dram_tensor("bias", [P, KT], F32, kind="ExternalInput").ap()
    Out = nc.dram_tensor("out", [S, H], F32, kind="ExternalOutput").ap()

    with tile.TileContext(nc) as tc, ExitStack() as root:
        const = root.enter_context(tc.tile_pool(name="const", bufs=1))

        ident_f = const.tile([P, P], F32, tag="idf")
        make_identity(nc, ident_f[:])
        ident_r = const.tile([P, P], F32R, tag="idr")
        nc.vector.tensor_copy(ident_r[:], ident_f[:])
        ident_b = const.tile([P, P], BF16, tag="idb")
        nc.vector.tensor_copy(ident_b[:], ident_f[:])
        ones_f = const.tile([P, 4], F32, tag="onesf")
        nc.vector.memset(ones_f[:], 1.0)
        ones_b = const.tile([P, 4], BF16, tag="ones")
        nc.vector.tensor_copy(ones_b[:], ones_f[:])
        bias_cols = const.tile([P, KT], F32, tag="bias")
        nc.sync.dma_start(bias_cols[:], Bias[:])

        # long-lived tensors
        c_pool = root.enter_context(tc.tile_pool(name="c", bufs=1))
        v_pool = root.enter_context(tc.tile_pool(name="v", bufs=1))

        with ExitStack() as ph1:
            mk_pool = ph1.enter_context(tc.tile_pool(name="mkq", bufs=1))
            m_tiles = []
            # phase 1a: G_kq[b,a] = sum_o Wk[o,b] Wq[o,a], plus Xq transposes
            with ExitStack() as s1a:
                w_pool = s1a.enter_context(tc.tile_pool(name="wkq", bufs=1))
                wqh_pool = s1a.enter_context(tc.tile_pool(name="wqh", bufs=1))
                ps1 = s1a.enter_context(tc.tile_pool(name="ps1a", bufs=1, space="PSUM"))
                wk_t = []
                for o in range(HT):
                    tk = w_pool.tile([P, H], F32R, name=f"wk{o}", tag=f"wk{o}")
                    nc.gpsimd.dma_start(tk[:], Wk[o * P:(o + 1) * P, :])
                    wk_t.append(tk)
                for b in range(HT):
                    mt = mk_pool.tile([P, H], F32R, name=f"m{b}", tag=f"m{b}")
                    m_tiles.append(mt)
                # o-major: first matmul only needs wk[0]+wq-slice[0], so PE
                # starts ~2.5us into the weight DMA instead of after all 8MB.
                # 8 PSUM banks hold the 8 b-groups of one column half.
                for ach in range(NCH):
                    wq_t = []
                    for o in range(HT):
                        tq = wqh_pool.tile([P, 512], F32R, tag=f"wq{o}")
                        nc.gpsimd.dma_start(
                            tq[:], Wq[o * P:(o + 1) * P, ach * 512:(ach + 1) * 512])
                        wq_t.append(tq)
                    pss = [ps1.tile([P, 512], F32, name=f"g{ach}_{b}", tag=f"mm{b}")
                           for b in range(HT)]
                    for o in range(HT):
                        for b in range(HT):
                            nc.tensor.matmul(
                                pss[b][:], wk_t[o][:, b * P:(b + 1) * P], wq_t[o][:],
                                start=(o == 0), stop=(o == HT - 1))
                    for b in range(HT):
                        if b % 2 == 0:
                            nc.vector.tensor_copy(
                                m_tiles[b][:, ach * 512:(ach + 1) * 512], pss[b][:])
                        else:
                            nc.scalar.copy(
                                m_tiles[b][:, ach * 512:(ach + 1) * 512], pss[b][:])

            # phase 1b: C[a,j] = sum_b G[b,a] XkT[b,j] over compacted keys,
            # interleaved with the Xq transposes (xqt resident for phase 2).
            # xqt_pool is entered on the root stack HERE (after the Wq/Wk
            # pool exits) so its 64KB never coexists with the weights.
            c_tiles = [c_pool.tile([P, Spad], F32R, name=f"c{a}", tag=f"c{a}")
                       for a in range(HT)]
            xqt = [c_pool.tile([P, S], F32R, name=f"xqt{h}", tag=f"xqt{h}")
                   for h in range(HT)]
            with ExitStack() as ph1b:
                xs_pool = ph1b.enter_context(tc.tile_pool(name="xk", bufs=4))
                xkt_pool = ph1b.enter_context(tc.tile_pool(name="xkt", bufs=2))
                pst = ph1b.enter_context(tc.tile_pool(name="ps1bt", bufs=4, space="PSUM"))
                psc = ph1b.enter_context(tc.tile_pool(name="ps1bc", bufs=2, space="PSUM"))
                jbs = []
                j0 = 0
                while j0 < KT:
                    nt = min(4, KT - j0)
                    jbs.append((j0, nt))
                    j0 += nt
                NQB = S // P          # 16 query row-blocks to transpose
                qb_next = 0

                def xq_transpose_block(it):
                    xn = xs_pool.tile([P, H], F32R, tag="xn")
                    nc.sync.dma_start(xn[:], Xq[it * P:(it + 1) * P, :])
                    for h in range(HT):
                        pt = pst.tile([P, P], F32R, tag="t")
                        nc.tensor.transpose(pt[:], xn[:, h * P:(h + 1) * P], ident_r[:])
                        if h % 2 == 0:
                            nc.vector.tensor_copy(xqt[h][:, it * P:(it + 1) * P], pt[:])
                        else:
                            nc.scalar.copy(xqt[h][:, it * P:(it + 1) * P], pt[:])

                for ci, (j0, nt) in enumerate(jbs):
                    JBw = nt * P
                    xkt = [xkt_pool.tile([P, JBw], F32R, name=f"xkt{h}", tag=f"xkt{h}")
                           for h in range(HT)]
                    for jt in range(nt):
                        xn = xs_pool.tile([P, H], F32R, tag="xn")
                        nc.sync.dma_start(
                            xn[:], Xk[(j0 + jt) * P:(j0 + jt + 1) * P, :])
                        for h in range(HT):
                            pt = pst.tile([P, P], F32R, tag="t")
                            nc.tensor.transpose(pt[:], xn[:, h * P:(h + 1) * P], ident_r[:])
                            if h % 2 == 0:
                                nc.vector.tensor_copy(xkt[h][:, jt * P:(jt + 1) * P], pt[:])
                            else:
                                nc.scalar.copy(xkt[h][:, jt * P:(jt + 1) * P], pt[:])
                    for a in range(HT):
                        ps = psc.tile([P, JBw], F32, tag="mm")
                        for b in range(HT):
                            nc.tensor.matmul(
                                ps[:], m_tiles[b][:, a * P:(a + 1) * P], xkt[b][:],
                                start=(b == 0), stop=(b == HT - 1))
                        nc.vector.tensor_copy(c_tiles[a][:, j0 * P:j0 * P + JBw], ps[:])
                    # spread the 16 Xq transpose blocks across the jb chunks
                    n_after = (NQB * (ci + 1)) // len(jbs)
                    while qb_next < n_after:
                        xq_transpose_block(qb_next)
                        qb_next += 1

        # phase 1c: v[j,:] = sum_h XvT[h,j].T @ WvT[h,:]  (bf16 operands)
        v_tiles = [v_pool.tile([P, H], BF16, name=f"v{j}", tag=f"v{j}")
                   for j in range(KT)]
        with ExitStack() as ph1c:
            wv_pool = ph1c.enter_context(tc.tile_pool(name="wvT", bufs=1))
            wb_pool = ph1c.enter_context(tc.tile_pool(name="wvb", bufs=3))
            xb_pool = ph1c.enter_context(tc.tile_pool(name="xvb", bufs=3))
            xvt_pool = ph1c.enter_context(tc.tile_pool(name="xvt", bufs=1))
            pst = ph1c.enter_context(tc.tile_pool(name="ps1ct", bufs=2, space="PSUM"))
            psv = ph1c.enter_context(tc.tile_pool(name="ps1cv", bufs=2, space="PSUM"))
            wvT = [wv_pool.tile([P, H], BF16, name=f"wvT{h}", tag=f"wvT{h}")
                   for h in range(HT)]
            for ot in range(HT):
                wb = wb_pool.tile([P, H], BF16, tag="wb")
                nc.gpsimd.dma_start(wb[:], Wv[ot * P:(ot + 1) * P, :])
                for h in range(HT):
                    pt = pst.tile([P, P], BF16, tag="t")
                    nc.tensor.transpose(pt[:], wb[:, h * P:(h + 1) * P], ident_b[:])
                    if h % 2 == 0:
                        nc.vector.tensor_copy(wvT[h][:, ot * P:(ot + 1) * P], pt[:])
                    else:
                        nc.scalar.copy(wvT[h][:, ot * P:(ot + 1) * P], pt[:])
            # all Xv transposes first (resident), then a dense v matmul stream
            xvt = [[xvt_pool.tile([P, P], BF16, name=f"xvt{kt}_{h}",
                                  tag=f"xvt{kt}_{h}") for h in range(HT)]
                   for kt in range(KT)]
            for kt in range(KT):
                xb = xb_pool.tile([P, H], BF16, tag="xb")
                nc.gpsimd.dma_start(xb[:], Xv[kt * P:(kt + 1) * P, :])
                for h in range(HT):
                    pt = pst.tile([P, P], BF16, tag="t")
                    nc.tensor.transpose(pt[:], xb[:, h * P:(h + 1) * P], ident_b[:])
                    if h % 2 == 0:
                        nc.vector.tensor_copy(xvt[kt][h][:], pt[:])
                    else:
                        nc.scalar.copy(xvt[kt][h][:], pt[:])
            for kt in range(KT):
                vps = [psv.tile([P, 512], F32, name=f"vps{ch}", tag=f"mm{ch}")
                       for ch in range(NCH)]
                for h in range(HT):
                    for ch in range(NCH):
                        nc.tensor.matmul(
                            vps[ch][:], xvt[kt][h][:], wvT[h][:, ch * 512:(ch + 1) * 512],
                            start=(h == 0), stop=(h == HT - 1))
                for ch in range(NCH):
                    nc.vector.tensor_copy(v_tiles[kt][:, ch * 512:(ch + 1) * 512], vps[ch][:])

        # phase 2: attention per i-block
        with ExitStack() as ph2:
            e_pool = ph2.enter_context(tc.tile_pool(name="expT", bufs=4))
            o_pool = ph2.enter_context(tc.tile_pool(name="ctxo", bufs=4))
            rec_pool = ph2.enter_context(tc.tile_pool(name="rec", bufs=2))
            ps_sc = ph2.enter_context(tc.tile_pool(name="ps2s", bufs=2, space="PSUM"))
            ps_ctx = ph2.enter_context(tc.tile_pool(name="ps2c", bufs=1, space="PSUM"))
            ps_z = ph2.enter_context(tc.tile_pool(name="ps2z", bufs=1, space="PSUM"))

            for ib in range(NIB):
                ctx_ps = [ps_ctx.tile([P, H], F32, name=f"ctx{i}", tag=f"ctx{i}")
                          for i in range(ISUB)]
                z_ps = ps_z.tile([P, 8], F32, tag="z")

                for jt in range(KT):
                    ps = ps_sc.tile([P, IB], F32, tag="sc")
                    for a in range(HT):
                        nc.tensor.matmul(
                            ps[:], c_tiles[a][:, jt * P:(jt + 1) * P],
                            xqt[a][:, ib * IB:(ib + 1) * IB],
                            start=(a == 0), stop=(a == HT - 1))
                    et = e_pool.tile([P, IB], BF16, tag="e")
                    nc.scalar.activation(et[:], ps[:], EXP,
                                         bias=bias_cols[:, jt:jt + 1], scale=1.0)
                    for isub in range(ISUB):
                        lhs = et[:, isub * P:(isub + 1) * P]
                        for ch in range(NCH):
                            nc.tensor.matmul(
                                ctx_ps[isub][:, ch * 512:(ch + 1) * 512],
                                lhs, v_tiles[jt][:, ch * 512:(ch + 1) * 512],
                                start=(jt == 0), stop=(jt == KT - 1))
                        nc.tensor.matmul(
                            z_ps[:, isub * 4:(isub + 1) * 4], lhs, ones_b[:],
                            start=(jt == 0 and isub == 0),
                            stop=(jt == KT - 1 and isub == ISUB - 1))

                rec = rec_pool.tile([P, ISUB], F32, tag="rec")
                nc.vector.reciprocal(rec[:], z_ps[:, 0:4 * ISUB:4])
                for isub in range(ISUB):
                    ot = o_pool.tile([P, H], F32, tag="o")
                    nc.vector.tensor_scalar_mul(ot[:], ctx_ps[isub][:], rec[:, isub:isub + 1])
                    nc.sync.dma_start(
                        Out[ib * IB + isub * P:ib * IB + (isub + 1) * P, :], ot[:])

    nc.compile()
    return nc


class _Runner:
    """Persistent PJRT executor mirroring bass2jax.run_bass_via_pjrt, built
    once so repeat kernel() calls skip jax retracing."""

    def __init__(self, nc, n_cores):
        import jax
        from jax.sharding import Mesh, PartitionSpec, NamedSharding
        from jax.experimental.shard_map import shard_map
        import concourse.mybir as mybir
        from concourse import bass2jax
        from concourse.bass2jax import _bass_exec_p, install_neuronx_cc_hook

        install_neuronx_cc_hook()
        self.jax = jax
        self.nc = nc
        self.n_cores = n_cores
        partition_name = (nc.partition_id_tensor.name
                          if nc.partition_id_tensor else None)
        in_names, out_names, out_avals = [], [], []
        for alloc in nc.m.functions[0].allocations:
            if not isinstance(alloc, mybir.MemoryLocationSet):
                continue
            name = alloc.memorylocations[0].name
            if alloc.kind == "ExternalInput":
                if name != partition_name:
                    in_names.append(name)
            elif alloc.kind == "ExternalOutput":
                out_names.append(name)
                out_avals.append(jax.core.ShapedArray(
                    tuple(alloc.tensor_shape), mybir.dt.np(alloc.dtype)))
        self.in_names, self.out_names, self.out_avals = in_names, out_names, out_avals
        n_params, n_outs = len(in_names), len(out_avals)
        self.n_params = n_params
        all_names = list(in_names) + list(out_names)
        if partition_name is not None:
            all_names.append(partition_name)

        def _body(*args):
            operands = list(args)
            if partition_name is not None:
                operands.append(bass2jax.partition_id_tensor())
            return tuple(_bass_exec_p.bind(
                *operands,
                out_avals=tuple(out_avals),
                in_names=tuple(all_names),
                out_names=tuple(out_names),
                lowering_input_output_aliases=(),
                sim_require_finite=True,
                sim_require_nnan=True,
                nc=nc,
            ))

        devices = jax.devices()[:n_cores]
        assert len(devices) == n_cores, f"need {n_cores} neuron cores"
        mesh = Mesh(np.asarray(devices), ("core",))
        in_specs = (PartitionSpec("core"),) * (n_params + n_outs)
        out_specs = (PartitionSpec("core"),) * n_outs
        donate = tuple(range(n_params, n_params + n_outs))
        self._fn = jax.jit(
            shard_map(_body, mesh=mesh, in_specs=in_specs,
                      out_specs=out_specs, check_rep=False),
            donate_argnums=donate, keep_unused=True)
        self.sharding = NamedSharding(mesh, PartitionSpec("core"))

    def run(self, in_maps):
        jax = self.jax
        in_arrs = [
            jax.device_put(
                np.concatenate([np.ascontiguousarray(m[n]) for m in in_maps], axis=0),
                self.sharding)
            for n in self.in_names
        ]
        zeros = [
            jax.device_put(
                np.zeros((self.n_cores * a.shape[0], *a.shape[1:]), a.dtype),
                self.sharding)
            for a in self.out_avals
        ]
        outs = self._fn(*in_arrs, *zeros)
        res = []
        for c in range(self.n_cores):
            res.append({
                n: np.asarray(outs[i]).reshape(self.n_cores, *self.out_avals[i].shape)[c]
                for i, n in enumerate(self.out_names)})
        return res


def _get_runner(KT):
    key = ("runner", KT)
    if key not in _CACHE:
        nc = _build(KT)
        _CACHE[key] = _Runner(nc, 8)
    return _CACHE[key]


def _make_in_maps(query, key, value, Wq, Wk, Wv, mask, KT, idxs):
    Spad = KT * P
    in_maps = []
    for c in range(B):
        idx = idxs[c]
        n = len(idx)
        kg = np.zeros((Spad, H), np.float32)
        vg = np.zeros((Spad, H), np.float32)
        if n:
            kg[:n] = key[c][idx]
            vg[:n] = value[c][idx]
        bias = np.full((Spad,), -50000.0, np.float32)
        bias[:n] = 0.0
        bias2d = np.ascontiguousarray(bias.reshape(KT, P).T)
        in_maps.append({
            "query": query[c], "key": kg, "value": vg,
            "Wq": Wq, "Wk": Wk, "Wv": Wv, "bias": bias2d,
        })
    return in_maps


def kernel(query, key, value, Wq, Wk, Wv, mask):
    query = np.asarray(query, dtype=np.float32)
    key = np.asarray(key, dtype=np.float32)
    value = np.asarray(value, dtype=np.float32)
    Wq = np.asarray(Wq, dtype=np.float32)
    Wk = np.asarray(Wk, dtype=np.float32)
    Wv = np.asarray(Wv, dtype=np.float32)
    mask = np.asarray(mask, dtype=np.int32)

    idxs = [np.flatnonzero(mask[c]) for c in range(B)]
    KT = max(1, (max(len(i) for i in idxs) + P - 1) // P)

    r = _get_runner(KT)
    in_maps = _make_in_maps(query, key, value, Wq, Wk, Wv, mask, KT, idxs)
    res = r.run(in_maps)
    out = np.stack([res[c]["out"] for c in range(B)])

    # a batch with every key masked: reference softmax is uniform over all
    # keys (all scores equal -99999), so ctx = mean(v) for every query row
    for c in range(B):
        if len(idxs[c]) == 0:
            v_mean = (value[c].mean(0) @ Wv.T).astype(np.float32)
            out[c][:] = v_mean[None, :]
    return out


# revision 34
# speedup vs baseline: 192.9094x; 1.0294x over previous
"""Trainium2 Bass kernel for nn_Attention_43628277793473.

Single-head attention, B=8, S=2048, H=1024:
  q = query @ Wq.T ; k = key @ Wk.T ; v = value @ Wv.T
  score = q @ k.T ; masked_fill(mask==0, -99999) ; softmax ; out = attn @ v

Sharding: data-parallel over the batch dim - one batch element per
NeuronCore (8 cores), no collectives.

Host-side key compaction: masked keys contribute exactly zero
(exp(score - 50000) underflows to 0.0 in f32, matching the reference
where exp(-99999 - rowmax) underflows), so kernel() gathers only the
unmasked key/value rows per batch, pads to a multiple of 128, and sends
a per-key bias column (0 for real keys, -50000 for padding). With the
~50% random mask this halves the score/context/projection work.

Per-core algorithm (scores in float32r, context path in bfloat16):
  G[b,a]   = sum_o Wk[o,b] Wq[o,a]       # weight gram, natural layouts
  xqt      = Xq^T tiles (PE transpose, f32r) for all 2048 queries
  C[a,j]   = sum_b G[b,a] XkT[b,j]       # keys compacted to KT*128
  v[j,:]   = XvT.T @ WvT                 # bf16 operands, f32 PSUM
  scoreT   = C-tile.T @ xqt-slice        # keys on partitions
  expT     = exp(scoreT + bias_col)      # -> bf16
  ctx[i,:] += expT[:,i].T @ v_j          # bf16 matmuls, f32 PSUM
  Z[i]     += expT[:,i].T @ ones_bf16
  out[i,:] = ctx[i,:] * (1/Z[i])
"""
import os
from contextlib import ExitStack

import numpy as np

_CACHE = {}

B, S, H, P = 8, 2048, 1024, 128


def _build(KT):
    import concourse.tile as tile
    from concourse import bacc, mybir
    from concourse.masks import make_identity

    F32 = mybir.dt.float32
    F32R = mybir.dt.float32r
    BF16 = mybir.dt.bfloat16
    EXP = mybir.ActivationFunctionType.Exp

    HT = H // P              # 8 hidden tiles
    Spad = KT * P            # compacted+padded key count
    IB = 256                 # queries per i-block
    NIB = S // IB
    ISUB = IB // P
    NCH = H // 512

    nc = bacc.Bacc("TRN2", target_bir_lowering=False, debug=False,
                   num_devices=8)

    Xq = nc.dram_tensor("query", [S, H], F32R, kind="ExternalInput").ap()
    Xk = nc.dram_tensor("key", [Spad, H], F32R, kind="ExternalInput").ap()
    Xv = nc.dram_tensor("value", [Spad, H], F32, kind="ExternalInput").ap()
    Wq = nc.dram_tensor("Wq", [H, H], F32, kind="ExternalInput").ap()
    Wk = nc.dram_tensor("Wk", [H, H], F32, kind="ExternalInput").ap()
    Wv = nc.dram_tensor("Wv", [H, H], F32, kind="ExternalInput").ap()
    Bias = nc.dram_tensor("bias", [P, KT], F32, kind="ExternalInput").ap()
    Out = nc.dram_tensor("out", [S, H], F32, kind="ExternalOutput").ap()

    with tile.TileContext(nc) as tc, ExitStack() as root:
        const = root.enter_context(tc.tile_pool(name="const", bufs=1))

        ident_f = const.tile([P, P], F32, tag="idf")
        make_identity(nc, ident_f[:])
        ident_r = const.tile([P, P], F32R, tag="idr")
        nc.vector.tensor_copy(ident_r[:], ident_f[:])
        ident_b = const.tile([P, P], BF16, tag="idb")
        nc.vector.tensor_copy(ident_b[:], ident_f[:])
        ones_f = const.tile([P, 4], F32, tag="onesf")
        nc.vector.memset(ones_f[:], 1.0)
        ones_b = const.tile([P, 4], BF16, tag="ones")
        nc.vector.tensor_copy(ones_b[:], ones_f[:])
        bias_cols = const.tile([P, KT], F32, tag="bias")
        nc.sync.dma_start(bias_cols[:], Bias[:])

        # long-lived tensors
        c_pool = root.enter_context(tc.tile_pool(name="c", bufs=1))
        v_pool = root.enter_context(tc.tile_pool(name="v", bufs=1))

        with ExitStack() as ph1:
            mk_pool = ph1.enter_context(tc.tile_pool(name="mkq", bufs=1))
            m_tiles = []
            # phase 1a: G_kq[b,a] = sum_o Wk[o,b] Wq[o,a], plus Xq transposes
            with ExitStack() as s1a:
                w_pool = s1a.enter_context(tc.tile_pool(name="wkq", bufs=1))
                wqh_pool = s1a.enter_context(tc.tile_pool(name="wqh", bufs=1))
                ps1 = s1a.enter_context(tc.tile_pool(name="ps1a", bufs=1, space="PSUM"))
                # o-major with interleaved wk/wq DMA issue: the first matmul
                # only needs wk[0]+wq-slice[0] (~0.75MB), so PE starts a few
                # us into the weight DMA instead of after all 8MB. 8 PSUM
                # banks hold the 8 b-groups of one column half.
                wk_t, wq0_t = [], []
                for o in range(HT):
                    tk = w_pool.tile([P, H], F32R, name=f"wk{o}", tag=f"wk{o}")
                    nc.gpsimd.dma_start(tk[:], Wk[o * P:(o + 1) * P, :])
                    wk_t.append(tk)
                    tq = wqh_pool.tile([P, 512], F32R, name=f"wqa{o}", tag=f"wq{o}")
                    nc.gpsimd.dma_start(tq[:], Wq[o * P:(o + 1) * P, 0:512])
                    wq0_t.append(tq)
                for b in range(HT):
                    mt = mk_pool.tile([P, H], F32R, name=f"m{b}", tag=f"m{b}")
                    m_tiles.append(mt)
                for ach in range(NCH):
                    if ach == 0:
                        wq_t = wq0_t
                    else:
                        wq_t = []
                        for o in range(HT):
                            tq = wqh_pool.tile([P, 512], F32R, name=f"wqb{o}",
                                               tag=f"wq{o}")
                            nc.gpsimd.dma_start(
                                tq[:], Wq[o * P:(o + 1) * P, ach * 512:(ach + 1) * 512])
                            wq_t.append(tq)
                    pss = [ps1.tile([P, 512], F32, name=f"g{ach}_{b}", tag=f"mm{b}")
                           for b in range(HT)]
                    for o in range(HT):
                        for b in range(HT):
                            nc.tensor.matmul(
                                pss[b][:], wk_t[o][:, b * P:(b + 1) * P], wq_t[o][:],
                                start=(o == 0), stop=(o == HT - 1))
                    for b in range(HT):
                        if b % 2 == 0:
                            nc.vector.tensor_copy(
                                m_tiles[b][:, ach * 512:(ach + 1) * 512], pss[b][:])
                        else:
                            nc.scalar.copy(
                                m_tiles[b][:, ach * 512:(ach + 1) * 512], pss[b][:])

            # phase 1b: C[a,j] = sum_b G[b,a] XkT[b,j] over compacted keys,
            # interleaved with the Xq transposes (xqt resident for phase 2).
            # xqt_pool is entered on the root stack HERE (after the Wq/Wk
            # pool exits) so its 64KB never coexists with the weights.
            c_tiles = [c_pool.tile([P, Spad], F32R, name=f"c{a}", tag=f"c{a}")
                       for a in range(HT)]
            xqt = [c_pool.tile([P, S], F32R, name=f"xqt{h}", tag=f"xqt{h}")
                   for h in range(HT)]
            with ExitStack() as ph1b:
                xs_pool = ph1b.enter_context(tc.tile_pool(name="xk", bufs=4))
                xkt_pool = ph1b.enter_context(tc.tile_pool(name="xkt", bufs=2))
                pst = ph1b.enter_context(tc.tile_pool(name="ps1bt", bufs=4, space="PSUM"))
                psc = ph1b.enter_context(tc.tile_pool(name="ps1bc", bufs=2, space="PSUM"))
                jbs = []
                j0 = 0
                while j0 < KT:
                    nt = min(4, KT - j0)
                    jbs.append((j0, nt))
                    j0 += nt
                NQB = S // P          # 16 query row-blocks to transpose
                qb_next = 0

                def xq_transpose_block(it):
                    xn = xs_pool.tile([P, H], F32R, tag="xn")
                    nc.sync.dma_start(xn[:], Xq[it * P:(it + 1) * P, :])
                    for h in range(HT):
                        pt = pst.tile([P, P], F32R, tag="t")
                        nc.tensor.transpose(pt[:], xn[:, h * P:(h + 1) * P], ident_r[:])
                        if h % 2 == 0:
                            nc.vector.tensor_copy(xqt[h][:, it * P:(it + 1) * P], pt[:])
                        else:
                            nc.scalar.copy(xqt[h][:, it * P:(it + 1) * P], pt[:])

                for ci, (j0, nt) in enumerate(jbs):
                    JBw = nt * P
                    xkt = [xkt_pool.tile([P, JBw], F32R, name=f"xkt{h}", tag=f"xkt{h}")
                           for h in range(HT)]
                    for jt in range(nt):
                        xn = xs_pool.tile([P, H], F32R, tag="xn")
                        nc.sync.dma_start(
                            xn[:], Xk[(j0 + jt) * P:(j0 + jt + 1) * P, :])
                        for h in range(HT):
                            pt = pst.tile([P, P], F32R, tag="t")
                            nc.tensor.transpose(pt[:], xn[:, h * P:(h + 1) * P], ident_r[:])
                            if h % 2 == 0:
                                nc.vector.tensor_copy(xkt[h][:, jt * P:(jt + 1) * P], pt[:])
                            else:
                                nc.scalar.copy(xkt[h][:, jt * P:(jt + 1) * P], pt[:])
                    for a in range(HT):
                        ps = psc.tile([P, JBw], F32, tag="mm")
                        for b in range(HT):
                            nc.tensor.matmul(
                                ps[:], m_tiles[b][:, a * P:(a + 1) * P], xkt[b][:],
                                start=(b == 0), stop=(b == HT - 1))
                        nc.vector.tensor_copy(c_tiles[a][:, j0 * P:j0 * P + JBw], ps[:])
                    # spread the 16 Xq transpose blocks across the jb chunks
                    n_after = (NQB * (ci + 1)) // len(jbs)
                    while qb_next < n_after:
                        xq_transpose_block(qb_next)
                        qb_next += 1

        # phase 1c: v[j,:] = sum_h XvT[h,j].T @ WvT[h,:]  (bf16 operands)
        v_tiles = [v_pool.tile([P, H], BF16, name=f"v{j}", tag=f"v{j}")
                   for j in range(KT)]
        with ExitStack() as ph1c:
            wv_pool = ph1c.enter_context(tc.tile_pool(name="wvT", bufs=1))
            wb_pool = ph1c.enter_context(tc.tile_pool(name="wvb", bufs=3))
            xb_pool = ph1c.enter_context(tc.tile_pool(name="xvb", bufs=3))
            xvt_pool = ph1c.enter_context(tc.tile_pool(name="xvt", bufs=1))
            pst = ph1c.enter_context(tc.tile_pool(name="ps1ct", bufs=2, space="PSUM"))
            psv = ph1c.enter_context(tc.tile_pool(name="ps1cv", bufs=2, space="PSUM"))
            wvT = [wv_pool.tile([P, H], BF16, name=f"wvT{h}", tag=f"wvT{h}")
                   for h in range(HT)]
            for ot in range(HT):
                wb = wb_pool.tile([P, H], BF16, tag="wb")
                nc.gpsimd.dma_start(wb[:], Wv[ot * P:(ot + 1) * P, :])
                for h in range(HT):
                    pt = pst.tile([P, P], BF16, tag="t")
                    nc.tensor.transpose(pt[:], wb[:, h * P:(h + 1) * P], ident_b[:])
                    if h % 2 == 0:
                        nc.vector.tensor_copy(wvT[h][:, ot * P:(ot + 1) * P], pt[:])
                    else:
                        nc.scalar.copy(wvT[h][:, ot * P:(ot + 1) * P], pt[:])
            # all Xv transposes first (resident), then a dense v matmul stream
            xvt = [[xvt_pool.tile([P, P], BF16, name=f"xvt{kt}_{h}",
                                  tag=f"xvt{kt}_{h}") for h in range(HT)]
                   for kt in range(KT)]
            for kt in range(KT):
                xb = xb_pool.tile([P, H], BF16, tag="xb")
                nc.gpsimd.dma_start(xb[:], Xv[kt * P:(kt + 1) * P, :])
                for h in range(HT):
                    pt = pst.tile([P, P], BF16, tag="t")
                    nc.tensor.transpose(pt[:], xb[:, h * P:(h + 1) * P], ident_b[:])
                    if h % 2 == 0:
                        nc.vector.tensor_copy(xvt[kt][h][:], pt[:])
                    else:
                        nc.scalar.copy(xvt[kt][h][:], pt[:])
            for kt in range(KT):
                vps = [psv.tile([P, 512], F32, name=f"vps{ch}", tag=f"mm{ch}")
                       for ch in range(NCH)]
                for h in range(HT):
                    for ch in range(NCH):
                        nc.tensor.matmul(
                            vps[ch][:], xvt[kt][h][:], wvT[h][:, ch * 512:(ch + 1) * 512],
                            start=(h == 0), stop=(h == HT - 1))
                for ch in range(NCH):
                    nc.vector.tensor_copy(v_tiles[kt][:, ch * 512:(ch + 1) * 512], vps[ch][:])

        # phase 2: attention per i-block
        with ExitStack() as ph2:
            e_pool = ph2.enter_context(tc.tile_pool(name="expT", bufs=4))
            o_pool = ph2.enter_context(tc.tile_pool(name="ctxo", bufs=4))
            rec_pool = ph2.enter_context(tc.tile_pool(name="rec", bufs=2))
            ps_sc = ph2.enter_context(tc.tile_pool(name="ps2s", bufs=2, space="PSUM"))
            ps_ctx = ph2.enter_context(tc.tile_pool(name="ps2c", bufs=1, space="PSUM"))
            ps_z = ph2.enter_context(tc.tile_pool(name="ps2z", bufs=1, space="PSUM"))

            for ib in range(NIB):
                ctx_ps = [ps_ctx.tile([P, H], F32, name=f"ctx{i}", tag=f"ctx{i}")
                          for i in range(ISUB)]
                z_ps = ps_z.tile([P, 8], F32, tag="z")

                for jt in range(KT):
                    ps = ps_sc.tile([P, IB], F32, tag="sc")
                    for a in range(HT):
                        nc.tensor.matmul(
                            ps[:], c_tiles[a][:, jt * P:(jt + 1) * P],
                            xqt[a][:, ib * IB:(ib + 1) * IB],
                            start=(a == 0), stop=(a == HT - 1))
                    et = e_pool.tile([P, IB], BF16, tag="e")
                    nc.scalar.activation(et[:], ps[:], EXP,
                                         bias=bias_cols[:, jt:jt + 1], scale=1.0)
                    for isub in range(ISUB):
                        lhs = et[:, isub * P:(isub + 1) * P]
                        for ch in range(NCH):
                            nc.tensor.matmul(
                                ctx_ps[isub][:, ch * 512:(ch + 1) * 512],
                                lhs, v_tiles[jt][:, ch * 512:(ch + 1) * 512],
                                start=(jt == 0), stop=(jt == KT - 1))
                        nc.tensor.matmul(
                            z_ps[:, isub * 4:(isub + 1) * 4], lhs, ones_b[:],
                            start=(jt == 0 and isub == 0),
                            stop=(jt == KT - 1 and isub == ISUB - 1))

                rec = rec_pool.tile([P, ISUB], F32, tag="rec")
                nc.vector.reciprocal(rec[:], z_ps[:, 0:4 * ISUB:4])
                for isub in range(ISUB):
                    ot = o_pool.tile([P, H], F32, tag="o")
                    nc.vector.tensor_scalar_mul(ot[:], ctx_ps[isub][:], rec[:, isub:isub + 1])
                    nc.sync.dma_start(
                        Out[ib * IB + isub * P:ib * IB + (isub + 1) * P, :], ot[:])

    nc.compile()
    return nc


class _Runner:
    """Persistent PJRT executor mirroring bass2jax.run_bass_via_pjrt, built
    once so repeat kernel() calls skip jax retracing."""

    def __init__(self, nc, n_cores):
        import jax
        from jax.sharding import Mesh, PartitionSpec, NamedSharding
        from jax.experimental.shard_map import shard_map
        import concourse.mybir as mybir
        from concourse import bass2jax
        from concourse.bass2jax import _bass_exec_p, install_neuronx_cc_hook

        install_neuronx_cc_hook()
        self.jax = jax
        self.nc = nc
        self.n_cores = n_cores
        partition_name = (nc.partition_id_tensor.name
                          if nc.partition_id_tensor else None)
        in_names, out_names, out_avals = [], [], []
        for alloc in nc.m.functions[0].allocations:
            if not isinstance(alloc, mybir.MemoryLocationSet):
                continue
            name = alloc.memorylocations[0].name
            if alloc.kind == "ExternalInput":
                if name != partition_name:
                    in_names.append(name)
            elif alloc.kind == "ExternalOutput":
                out_names.append(name)
                out_avals.append(jax.core.ShapedArray(
                    tuple(alloc.tensor_shape), mybir.dt.np(alloc.dtype)))
        self.in_names, self.out_names, self.out_avals = in_names, out_names, out_avals
        n_params, n_outs = len(in_names), len(out_avals)
        self.n_params = n_params
        all_names = list(in_names) + list(out_names)
        if partition_name is not None:
            all_names.append(partition_name)

        def _body(*args):
            operands = list(args)
            if partition_name is not None:
                operands.append(bass2jax.partition_id_tensor())
            return tuple(_bass_exec_p.bind(
                *operands,
                out_avals=tuple(out_avals),
                in_names=tuple(all_names),
                out_names=tuple(out_names),
                lowering_input_output_aliases=(),
                sim_require_finite=True,
                sim_require_nnan=True,
                nc=nc,
            ))

        devices = jax.devices()[:n_cores]
        assert len(devices) == n_cores, f"need {n_cores} neuron cores"
        mesh = Mesh(np.asarray(devices), ("core",))
        in_specs = (PartitionSpec("core"),) * (n_params + n_outs)
        out_specs = (PartitionSpec("core"),) * n_outs
        donate = tuple(range(n_params, n_params + n_outs))
        self._fn = jax.jit(
            shard_map(_body, mesh=mesh, in_specs=in_specs,
                      out_specs=out_specs, check_rep=False),
            donate_argnums=donate, keep_unused=True)
        self.sharding = NamedSharding(mesh, PartitionSpec("core"))

    def run(self, in_maps):
        jax = self.jax
        in_arrs = [
            jax.device_put(
                np.concatenate([np.ascontiguousarray(m[n]) for m in in_maps], axis=0),
                self.sharding)
            for n in self.in_names
        ]
        zeros = [
            jax.device_put(
                np.zeros((self.n_cores * a.shape[0], *a.shape[1:]), a.dtype),
                self.sharding)
            for a in self.out_avals
        ]
        outs = self._fn(*in_arrs, *zeros)
        res = []
        for c in range(self.n_cores):
            res.append({
                n: np.asarray(outs[i]).reshape(self.n_cores, *self.out_avals[i].shape)[c]
                for i, n in enumerate(self.out_names)})
        return res


def _get_runner(KT):
    key = ("runner", KT)
    if key not in _CACHE:
        nc = _build(KT)
        _CACHE[key] = _Runner(nc, 8)
    return _CACHE[key]


def _make_in_maps(query, key, value, Wq, Wk, Wv, mask, KT, idxs):
    Spad = KT * P
    in_maps = []
    for c in range(B):
        idx = idxs[c]
        n = len(idx)
        kg = np.zeros((Spad, H), np.float32)
        vg = np.zeros((Spad, H), np.float32)
        if n:
            kg[:n] = key[c][idx]
            vg[:n] = value[c][idx]
        bias = np.full((Spad,), -50000.0, np.float32)
        bias[:n] = 0.0
        bias2d = np.ascontiguousarray(bias.reshape(KT, P).T)
        in_maps.append({
            "query": query[c], "key": kg, "value": vg,
            "Wq": Wq, "Wk": Wk, "Wv": Wv, "bias": bias2d,
        })
    return in_maps


def kernel(query, key, value, Wq, Wk, Wv, mask):
    query = np.asarray(query, dtype=np.float32)
    key = np.asarray(key, dtype=np.float32)
    value = np.asarray(value, dtype=np.float32)
    Wq = np.asarray(Wq, dtype=np.float32)
    Wk = np.asarray(Wk, dtype=np.float32)
    Wv = np.asarray(Wv, dtype=np.float32)
    mask = np.asarray(mask, dtype=np.int32)

    idxs = [np.flatnonzero(mask[c]) for c in range(B)]
    KT = max(1, (max(len(i) for i in idxs) + P - 1) // P)

    r = _get_runner(KT)
    in_maps = _make_in_maps(query, key, value, Wq, Wk, Wv, mask, KT, idxs)
    res = r.run(in_maps)
    out = np.stack([res[c]["out"] for c in range(B)])

    # a batch with every key masked: reference softmax is uniform over all
    # keys (all scores equal -99999), so ctx = mean(v) for every query row
    for c in range(B):
        if len(idxs[c]) == 0:
            v_mean = (value[c].mean(0) @ Wv.T).astype(np.float32)
            out[c][:] = v_mean[None, :]
    return out
